# revision 13
# baseline (speedup 1.0000x reference)
"""DecompAtt (decomposable attention) Trainium2 Bass kernel.

Full-input contract: kernel(**inputs) takes the unsharded fp32 inputs
(B=32, P=H=384, D=DA=DC=DH=512, 3 labels) and returns
(label_logits, label_probs, h2p, p2h) as full-shape np arrays.

Sharding: data-parallel over batch, 4 examples per core on 8 NeuronCores.

Numerics:
  - score path (attend FF + similarity) in tf32x2 (hi/lo split, 3 fp32r
    passes -> fp32-grade scores; measured 1.5e-7 rel on HW)
  - attention application + compare FF in single-pass fp32r (tf32)
  - aggregate MLP in true fp32 (it dominates logit error otherwise)
  - masked softmax with additive -1e30 mask folded into PSUM via K=1
    matmuls; exp on ACT with per-partition bias and fused row-sum.
"""

import numpy as np

import concourse.bacc as bacc
import concourse.mybir as mybir
import concourse.tile as tile
from concourse import masks

F32 = mybir.dt.float32
F32R = mybir.dt.float32r
AF = mybir.ActivationFunctionType
AX = mybir.AxisListType
OP = mybir.AluOpType

B, P, H, D = 32, 384, 384, 512
DA, DC, DH, NL = 512, 512, 512, 3
NCORES = 8
BE = B // NCORES  # examples per core
NEG = np.float32(-1.0e30)

_CACHE = {}


def r(ap):
    return ap.bitcast(F32R)


def _build(reps=1):
    nc = bacc.Bacc()

    # ---- per-core DRAM parameters
    emb_p = nc.declare_dram_parameter("emb_p", [BE, P, D], F32, isOutput=False)
    emb_h = nc.declare_dram_parameter("emb_h", [BE, H, D], F32, isOutput=False)
    pmadd = nc.declare_dram_parameter("pmadd", [BE, P], F32, isOutput=False)
    hmadd = nc.declare_dram_parameter("hmadd", [BE, H], F32, isOutput=False)
    pmf = nc.declare_dram_parameter("pmf", [BE, P], F32, isOutput=False)
    hmf = nc.declare_dram_parameter("hmf", [BE, H], F32, isOutput=False)
    w_att = nc.declare_dram_parameter("w_att", [D, DA], F32, isOutput=False)
    b_att = nc.declare_dram_parameter("b_att", [DA, 1], F32, isOutput=False)
    w_cmp = nc.declare_dram_parameter("w_cmp", [2 * D, DC], F32, isOutput=False)
    bc_row = nc.declare_dram_parameter("bc_row", [1, DC], F32, isOutput=False)
    w_ag1 = nc.declare_dram_parameter("w_ag1", [2 * DC, DH], F32, isOutput=False)
    b_ag1 = nc.declare_dram_parameter("b_ag1", [DH, 1], F32, isOutput=False)
    w_ag2 = nc.declare_dram_parameter("w_ag2", [DH, NL], F32, isOutput=False)
    b2_row = nc.declare_dram_parameter("b2_row", [1, NL], F32, isOutput=False)

    o_log = nc.declare_dram_parameter("o_log", [BE, NL], F32, isOutput=True)
    o_prb = nc.declare_dram_parameter("o_prb", [BE, NL], F32, isOutput=True)
    o_h2p = nc.declare_dram_parameter("o_h2p", [BE, H, P], F32, isOutput=True)
    o_p2h = nc.declare_dram_parameter("o_p2h", [BE, P, H], F32, isOutput=True)

    TOK = P  # == H == 384; 3 token tiles of 128
    NT = TOK // 128  # 3
    ND = D // 128    # 4

    with tile.TileContext(nc) as tc:
        with (
            tc.tile_pool(name="wp", bufs=1) as wp,
            tc.tile_pool(name="xp", bufs=1) as xp,
            tc.tile_pool(name="ps_mm", bufs=3, space="PSUM") as ps_mm,
            tc.tile_pool(name="ps_tr", bufs=3, space="PSUM") as ps_tr,
            tc.tile_pool(name="ps_sm", bufs=2, space="PSUM") as ps_sm,
        ):
            _rep = tc.For_i(0, reps, 1) if reps > 1 else None
            if _rep is not None:
                _rep.__enter__()
            # ================= constants & weights =================
            ident = wp.tile([128, 128], F32, name="ident")
            masks.make_identity(nc, ident[:])

            ones_raw = wp.tile([1, 128], F32, name="ones_raw")
            nc.vector.memset(ones_raw[:], 1.0)
            ones_r = wp.tile([1, 128], F32, name="ones_r")
            nc.vector.tensor_copy(r(ones_r[:]), ones_raw[:])
            ones4 = wp.tile([1, BE], F32, name="ones4")
            nc.vector.memset(ones4[:], 1.0)

            wa_hi, wa_lo, wcr, w1t, w2t = [], [], [], [], []
            for k in range(ND):
                t = xp.tile([128, DA], F32, name=f"wa_raw{k}", tag="wa_raw", bufs=2)
                nc.sync.dma_start(out=t[:], in_=w_att[128 * k:128 * (k + 1), :])
                hi = wp.tile([128, DA], F32, name=f"wa_hi{k}")
                lo = wp.tile([128, DA], F32, name=f"wa_lo{k}")
                nc.vector.tensor_copy(r(hi[:]), t[:])
                nc.vector.tensor_sub(r(lo[:]), t[:], hi[:])
                wa_hi.append(hi)
                wa_lo.append(lo)
            for k in range(2 * D // 128):
                t = xp.tile([128, DC], F32, name=f"wc_raw{k}", tag="wa_raw", bufs=2)
                nc.sync.dma_start(out=t[:], in_=w_cmp[128 * k:128 * (k + 1), :])
                t2 = wp.tile([128, DC], F32, name=f"wc{k}")
                nc.vector.tensor_copy(r(t2[:]), t[:])
                wcr.append(t2)
            def load_w1(k):
                t = xp.tile([128, DH], F32, name=f"w1_{k}", tag="wa_raw", bufs=2)
                nc.sync.dma_start(out=t[:], in_=w_ag1[128 * k:128 * (k + 1), :])
                return t
            for k in range(DH // 128):
                t = wp.tile([128, NL], F32, name=f"w2_{k}")
                nc.sync.dma_start(out=t[:], in_=w_ag2[128 * k:128 * (k + 1), :])
                w2t.append(t)

            batt = wp.tile([128, ND], F32, name="batt")
            nc.sync.dma_start(out=batt[:], in_=b_att[:].rearrange("(j p) o -> p (j o)", p=128))
            b1c = wp.tile([128, DH // 128], F32, name="b1c")
            nc.sync.dma_start(out=b1c[:], in_=b_ag1[:].rearrange("(j p) o -> p (j o)", p=128))
            bcr_raw = wp.tile([1, DC], F32, name="bcr_raw")
            nc.sync.dma_start(out=bcr_raw[:], in_=bc_row[:])
            bcr = wp.tile([1, DC], F32, name="bcr")
            nc.vector.tensor_copy(r(bcr[:]), bcr_raw[:])
            b2r = wp.tile([1, NL], F32, name="b2r")
            nc.sync.dma_start(out=b2r[:], in_=b2_row[:])

            # per-example additive mask rows, each at partition 0
            madd_p_e, madd_h_e = [], []
            for e in range(BE):
                tp0 = wp.tile([1, TOK], F32, name=f"maddp_raw{e}")
                nc.sync.dma_start(out=tp0[:], in_=pmadd[e:e + 1, :])
                tp = wp.tile([1, TOK], F32, name=f"maddp{e}")
                nc.vector.tensor_copy(r(tp[:]), tp0[:])
                madd_p_e.append(tp)
                th0 = wp.tile([1, TOK], F32, name=f"maddh_raw{e}")
                nc.sync.dma_start(out=th0[:], in_=hmadd[e:e + 1, :])
                th = wp.tile([1, TOK], F32, name=f"maddh{e}")
                nc.vector.tensor_copy(r(th[:]), th0[:])
                madd_h_e.append(th)

            # mask columns [128, NT] per example, per side
            pmc = wp.tile([128, BE * NT], F32, name="pmc")
            nc.sync.dma_start(out=pmc[:], in_=pmf[:].rearrange("e (t p) -> p (e t)", p=128))
            hmc = wp.tile([128, BE * NT], F32, name="hmc")
            nc.sync.dma_start(out=hmc[:], in_=hmf[:].rearrange("e (t p) -> p (e t)", p=128))

            # aggregate rhs: 8 k-tiles [128, BE]
            aggt = [wp.tile([128, BE], F32, name=f"agg{k}") for k in range(8)]

            # ================= helpers =================
            def softmax_from_psum(zp, out_sb, tag):
                """masked softmax over free dim of z (PSUM [128, TOK]) -> out_sb fp32."""
                negc = xp.tile([128, 1], F32, name=f"negc_{tag}", tag="negc", bufs=4)
                nc.vector.tensor_reduce(out=negc[:], in_=zp[:], axis=AX.X,
                                        op=OP.max, negate=True)
                ts = xp.tile([128, 1], F32, name=f"ts_{tag}", tag="ts", bufs=4)
                nc.scalar.activation(out_sb[:], zp[:], AF.Exp,
                                     bias=negc[:], scale=1.0, accum_out=ts[:])
                it = xp.tile([128, 1], F32, name=f"it_{tag}", tag="it", bufs=4)
                nc.vector.reciprocal(it[:], ts[:])
                nc.scalar.activation(out_sb[:], out_sb[:], AF.Copy, bias=0.0, scale=it[:])

            # ================= per-example pipeline =================
            for e in range(BE):
                # ---- load emb tiles (native [tok, D])
                embs = {}
                for side, src in (("p", emb_p), ("h", emb_h)):
                    tl = []
                    for t in range(NT):
                        s = xp.tile([128, D], F32, name=f"emb{side}{t}_e{e}",
                                    tag=f"emb{side}{t}", bufs=1)
                        nc.sync.dma_start(out=s[:], in_=src[e, 128 * t:128 * (t + 1), :])
                        tl.append(s)
                    embs[side] = tl

                # ---- embT (transpose) + hi/lo split : per side 4 tiles [128(d), TOK]
                ehi, elo = {}, {}
                for side in ("p", "h"):
                    his, los = [], []
                    for dj in range(ND):
                        pt = ps_tr.tile([128, TOK], F32, name=f"ptT{side}{dj}_e{e}",
                                        tag="ps_tr_big", bufs=3)
                        for ti in range(NT):
                            nc.tensor.matmul(
                                pt[:, 128 * ti:128 * (ti + 1)],
                                embs[side][ti][:, 128 * dj:128 * (dj + 1)],
                                ident[:], is_transpose=True,
                                start=(ti == 0), stop=(ti == NT - 1))
                        hi = xp.tile([128, TOK], F32, name=f"ehi{side}{dj}_e{e}",
                                     tag=f"ehi{side}{dj}", bufs=2)
                        lo = xp.tile([128, TOK], F32, name=f"elo{side}{dj}_e{e}",
                                     tag=f"elo{side}{dj}", bufs=1)
                        nc.scalar.activation(r(hi[:]), pt[:], AF.Copy)
                        nc.vector.tensor_sub(r(lo[:]), pt[:], hi[:])
                        his.append(hi)
                        los.append(lo)
                    ehi[side], elo[side] = his, los

                # ---- rounded copies of native emb (f32r) for M3 lhsT
                embr = {}
                for side in ("p", "h"):
                    rl = []
                    for t in range(NT):
                        rt = xp.tile([128, D], F32, name=f"embr{side}{t}_e{e}",
                                     tag=f"embr{side}{t}", bufs=1)
                        nc.vector.tensor_copy(r(rt[:]), embs[side][t][:])
                        rl.append(rt)
                    embr[side] = rl

                # ---- M1: projT = relu(W_att.T @ embT + b) ; tf32x2, keep hi/lo
                phi, plo = {}, {}
                for side in ("p", "h"):
                    his, los = [], []
                    for dj in range(ND):
                        pp = ps_mm.tile([128, TOK], F32, name=f"pp{side}{dj}_e{e}",
                                        tag="ps_mm_big", bufs=3)
                        sl = slice(128 * dj, 128 * (dj + 1))
                        for k in range(ND):
                            nc.tensor.matmul(pp[:], r(wa_hi[k][:, sl]), r(ehi[side][k][:]),
                                             start=(k == 0), stop=False)
                        for k in range(ND):
                            nc.tensor.matmul(pp[:], r(wa_hi[k][:, sl]), r(elo[side][k][:]),
                                             start=False, stop=False)
                        for k in range(ND):
                            nc.tensor.matmul(pp[:], r(wa_lo[k][:, sl]), r(ehi[side][k][:]),
                                             start=False, stop=(k == ND - 1))
                        fu = xp.tile([128, TOK], F32, name=f"pf{side}{dj}_e{e}",
                                     tag="proj_full", bufs=2)
                        nc.scalar.activation(fu[:], pp[:], AF.Relu,
                                             bias=batt[:, dj:dj + 1], scale=1.0)
                        hi = xp.tile([128, TOK], F32, name=f"phi{side}{dj}_e{e}",
                                     tag=f"phi{side}{dj}", bufs=1)
                        lo = xp.tile([128, TOK], F32, name=f"plo{side}{dj}_e{e}",
                                     tag=f"plo{side}{dj}", bufs=1)
                        nc.vector.tensor_copy(r(hi[:]), fu[:])
                        nc.vector.tensor_sub(r(lo[:]), fu[:], hi[:])
                        his.append(hi)
                        los.append(lo)
                    phi[side], plo[side] = his, los

                # ---- M2: sim tiles [128(p), H] tf32x2 ; raw copy; +hm mask; softmax
                sim_sb, p2h_sb = [], []
                for pi in range(NT):
                    sp = ps_mm.tile([128, TOK], F32, name=f"sim{pi}_e{e}",
                                    tag="ps_mm_big", bufs=3)
                    sl = slice(128 * pi, 128 * (pi + 1))
                    for k in range(ND):
                        nc.tensor.matmul(sp[:], r(phi["p"][k][:, sl]), r(phi["h"][k][:]),
                                         start=(k == 0), stop=False)
                    for k in range(ND):
                        nc.tensor.matmul(sp[:], r(phi["p"][k][:, sl]), r(plo["h"][k][:]),
                                         start=False, stop=False)
                    for k in range(ND):
                        nc.tensor.matmul(sp[:], r(plo["p"][k][:, sl]), r(phi["h"][k][:]),
                                         start=False, stop=False)
                    raw = xp.tile([128, TOK], F32, name=f"simsb{pi}_e{e}",
                                  tag=f"simsb{pi}", bufs=1)
                    nc.scalar.activation(raw[:], sp[:], AF.Copy)
                    sim_sb.append(raw)
                    nc.tensor.matmul(sp[:], r(ones_r[:]), r(madd_h_e[e][:]),
                                     start=False, stop=True)
                    o = xp.tile([128, TOK], F32, name=f"p2h{pi}_e{e}",
                                tag=f"p2h{pi}", bufs=1)
                    softmax_from_psum(sp, o, f"p2h{pi}e{e}")
                    nc.sync.dma_start(out=o_p2h[e, 128 * pi:128 * (pi + 1), :], in_=o[:])
                    p2h_sb.append(o)

                # ---- simT via PE transpose + pm mask; softmax -> h2p
                h2p_sb = []
                for hi_ in range(NT):
                    st = ps_tr.tile([128, TOK], F32, name=f"simT{hi_}_e{e}",
                                    tag="ps_tr_big", bufs=3)
                    for pi in range(NT):
                        nc.tensor.matmul(
                            st[:, 128 * pi:128 * (pi + 1)],
                            sim_sb[pi][:, 128 * hi_:128 * (hi_ + 1)],
                            ident[:], is_transpose=True,
                            start=(pi == 0), stop=False)
                    nc.tensor.matmul(st[:], r(ones_r[:]), r(madd_p_e[e][:]),
                                     start=False, stop=True)
                    o = xp.tile([128, TOK], F32, name=f"h2p{hi_}_e{e}",
                                tag=f"h2p{hi_}", bufs=1)
                    softmax_from_psum(st, o, f"h2p{hi_}e{e}")
                    nc.sync.dma_start(out=o_h2p[e, 128 * hi_:128 * (hi_ + 1), :], in_=o[:])
                    h2p_sb.append(o)

                # ---- transposes of p2h/h2p (f32r) for attended matmuls
                def transpose_set(src_tiles, tag):
                    outs = []
                    for j in range(NT):
                        tp = ps_tr.tile([128, TOK], F32, name=f"{tag}T{j}_e{e}",
                                        tag="ps_tr_big", bufs=3)
                        for i in range(NT):
                            nc.tensor.matmul(
                                tp[:, 128 * i:128 * (i + 1)],
                                src_tiles[i][:, 128 * j:128 * (j + 1)],
                                ident[:], is_transpose=True,
                                start=(i == 0), stop=(i == NT - 1))
                        ot = xp.tile([128, TOK], F32, name=f"{tag}{j}_e{e}",
                                     tag=f"{tag}{j}", bufs=1)
                        nc.scalar.activation(r(ot[:]), tp[:], AF.Copy)
                        outs.append(ot)
                    return outs

                p2hT = transpose_set(p2h_sb, "p2hT")  # [128(h), P] tiles
                h2pT = transpose_set(h2p_sb, "h2pT")  # [128(p), H] tiles

                # ---- M3: attended (tf32): attT_h [128(d), P] ; attT_p [128(d), H]
                def attend(embn, probT, tag):
                    outs = []
                    for dj in range(ND):
                        ap = ps_mm.tile([128, TOK], F32, name=f"{tag}{dj}_e{e}",
                                        tag="ps_mm_big", bufs=3)
                        sl = slice(128 * dj, 128 * (dj + 1))
                        for t in range(NT):
                            nc.tensor.matmul(ap[:], r(embn[t][:, sl]), r(probT[t][:]),
                                             start=(t == 0), stop=(t == NT - 1))
                        ot = xp.tile([128, TOK], F32, name=f"{tag}sb{dj}_e{e}",
                                     tag=f"{tag}{dj}", bufs=1)
                        nc.scalar.activation(r(ot[:]), ap[:], AF.Copy)
                        outs.append(ot)
                    return outs

                att_h = attend(embr["h"], p2hT, "atth")  # attended_hypothesis^T
                att_p = attend(embr["p"], h2pT, "attp")  # attended_premise^T

                # ---- M4: compare (token-major [128(tok), DC]) + masked reduce via PE
                def compare(concat_tiles, mcol_all, cmpd_ps, tag):
                    # cmpd_ps: PSUM [128, 4] accumulating masked sums (dc chunks as cols)
                    for ti in range(NT):
                        cp = ps_mm.tile([128, DC], F32, name=f"cp{tag}{ti}_e{e}",
                                        tag="ps_mm_big", bufs=3)
                        sl = slice(128 * ti, 128 * (ti + 1))
                        for k in range(8):
                            nc.tensor.matmul(cp[:], r(concat_tiles[k][:, sl]), r(wcr[k][:]),
                                             start=(k == 0), stop=False)
                        nc.tensor.matmul(cp[:], r(ones_r[:]), r(bcr[:]),
                                         start=False, stop=True)
                        cm = xp.tile([128, DC], F32, name=f"cm{tag}{ti}_e{e}",
                                     tag="cmp_scratch", bufs=2)
                        nc.scalar.activation(cm[:], cp[:], AF.Relu)
                        mc = mcol_all[:, (e * NT + ti):(e * NT + ti + 1)]
                        for j in range(4):
                            nc.tensor.matmul(
                                cmpd_ps[:, j:j + 1],
                                cm[:, 128 * j:128 * (j + 1)], mc,
                                start=(ti == 0 and j == 0),
                                stop=(ti == NT - 1 and j == 3))

                cdp = ps_sm.tile([128, 4], F32, name=f"cdp_e{e}", tag="smll", bufs=2)
                cdh = ps_sm.tile([128, 4], F32, name=f"cdh_e{e}", tag="smll", bufs=2)
                compare(ehi["p"] + att_h, pmc, cdp, "p")
                compare(ehi["h"] + att_p, hmc, cdh, "h")
                for j in range(4):
                    nc.vector.tensor_copy(aggt[j][:, e:e + 1], cdp[:, j:j + 1])
                    nc.vector.tensor_copy(aggt[4 + j][:, e:e + 1], cdh[:, j:j + 1])

            # ================= aggregate MLP (fp32), all examples =================
            nh = DH // 128
            hp = ps_sm.tile([128, BE * nh], F32, name="hid_all", tag="smll", bufs=2)
            for k in range(8):
                wk = load_w1(k)
                for dj in range(nh):
                    nc.tensor.matmul(hp[:, BE * dj:BE * (dj + 1)],
                                     wk[:, 128 * dj:128 * (dj + 1)], aggt[k][:],
                                     start=(k == 0 and dj == 0),
                                     stop=(k == 7 and dj == nh - 1))
            hid = []
            for dj in range(nh):
                hs = wp.tile([128, BE], F32, name=f"hids{dj}")
                nc.scalar.activation(hs[:], hp[:, BE * dj:BE * (dj + 1)], AF.Relu,
                                     bias=b1c[:, dj:dj + 1], scale=1.0)
                hid.append(hs)

            lp = ps_sm.tile([BE, NL], F32, name="logitp", tag="smll", bufs=2)
            for k in range(DH // 128):
                nc.tensor.matmul(lp[:], hid[k][:], w2t[k][:],
                                 start=(k == 0), stop=False)
            nc.tensor.matmul(lp[:], ones4[:], b2r[:], start=False, stop=True)
            lg = wp.tile([BE, NL], F32, name="lg")
            nc.vector.tensor_copy(lg[:], lp[:])
            nc.sync.dma_start(out=o_log[:], in_=lg[:])

            ng4 = wp.tile([BE, 1], F32, name="ng4")
            nc.vector.tensor_reduce(out=ng4[:], in_=lg[:], axis=AX.X, op=OP.max,
                                    negate=True)
            pr = wp.tile([BE, NL], F32, name="pr")
            t4 = wp.tile([BE, 1], F32, name="t4")
            nc.scalar.activation(pr[:], lg[:], AF.Exp, bias=ng4[:], scale=1.0,
                                 accum_out=t4[:])
            i4 = wp.tile([BE, 1], F32, name="i4")
            nc.vector.reciprocal(i4[:], t4[:])
            nc.scalar.activation(pr[:], pr[:], AF.Copy, bias=0.0, scale=i4[:])
            nc.sync.dma_start(out=o_prb[:], in_=pr[:])
            if _rep is not None:
                _rep.__exit__(None, None, None)

    nc.finalize()
    return nc


def _get_runner():
    """Build nc once, jit the sharded executor once; reuse across calls."""
    if "runner" in _CACHE:
        return _CACHE["runner"]

    import jax
    from jax.sharding import Mesh, PartitionSpec
    from jax.experimental.shard_map import shard_map
    from concourse import bass2jax

    nc = _build()
    bass2jax.install_neuronx_cc_hook()

    in_names = []
    out_names = []
    out_avals = []
    zero_outs = []
    partition_name = nc.partition_id_tensor.name if nc.partition_id_tensor else None
    for alloc in nc.m.functions[0].allocations:
        if not isinstance(alloc, mybir.MemoryLocationSet):
            continue
        name = alloc.memorylocations[0].name
        if alloc.kind == "ExternalInput":
            if name != partition_name:
                in_names.append(name)
        elif alloc.kind == "ExternalOutput":
            shape = tuple(alloc.tensor_shape)
            dtype = mybir.dt.np(alloc.dtype)
            out_names.append(name)
            out_avals.append(jax.core.ShapedArray(shape, dtype))
            zero_outs.append(np.zeros(shape, dtype))
    n_params = len(in_names)
    n_outs = len(out_names)
    all_in = in_names + out_names

    if partition_name is not None:
        all_in = all_in + [partition_name]

    def _body(*args):
        operands = list(args)
        if partition_name is not None:
            operands.append(bass2jax.partition_id_tensor())
        outs = bass2jax._bass_exec_p.bind(
            *operands,
            out_avals=tuple(out_avals),
            in_names=tuple(all_in),
            out_names=tuple(out_names),
            lowering_input_output_aliases=(),
            sim_require_finite=True,
            sim_require_nnan=True,
            nc=nc,
        )
        return tuple(outs)

    devices = jax.devices()[:NCORES]
    mesh = Mesh(np.asarray(devices), ("core",))
    in_specs = (PartitionSpec("core"),) * (n_params + n_outs)
    out_specs = (PartitionSpec("core"),) * n_outs
    donate = tuple(range(n_params, n_params + n_outs))
    sharded = jax.jit(
        shard_map(_body, mesh=mesh, in_specs=in_specs, out_specs=out_specs,
                  check_rep=False),
        donate_argnums=donate, keep_unused=True)

    def run(per_core_maps):
        concat_in = [
            np.concatenate([per_core_maps[c][n] for c in range(NCORES)], axis=0)
            for n in in_names
        ]
        concat_zero = [
            np.zeros((NCORES * z.shape[0], *z.shape[1:]), z.dtype) for z in zero_outs
        ]
        outs = sharded(*concat_in, *concat_zero)
        return {
            n: np.asarray(outs[i]).reshape(NCORES, *out_avals[i].shape)
            for i, n in enumerate(out_names)
        }

    _CACHE["nc"] = nc
    _CACHE["runner"] = run
    _CACHE["exec_parts"] = {
        "nc": nc, "in_names": in_names, "out_names": out_names,
        "out_avals": out_avals, "zero_outs": zero_outs,
        "partition_name": partition_name, "body": _body,
    }
    return run


def _prep_core_inputs(inputs, c):
    sl = slice(c * BE, (c + 1) * BE)
    ep = np.ascontiguousarray(inputs["embedded_premise"][sl]).astype(np.float32)
    eh = np.ascontiguousarray(inputs["embedded_hypothesis"][sl]).astype(np.float32)
    pm = inputs["premise_mask"][sl].astype(np.float32)
    hm = inputs["hypothesis_mask"][sl].astype(np.float32)
    return {
        "emb_p": ep,
        "emb_h": eh,
        "pmadd": np.where(pm > 0, np.float32(0), NEG).astype(np.float32),
        "hmadd": np.where(hm > 0, np.float32(0), NEG).astype(np.float32),
        "pmf": pm,
        "hmf": hm,
        "w_att": inputs["W_attend"].astype(np.float32),
        "b_att": inputs["b_attend"].astype(np.float32).reshape(DA, 1),
        "w_cmp": inputs["W_compare"].astype(np.float32),
        "bc_row": inputs["b_compare"].astype(np.float32).reshape(1, DC),
        "w_ag1": inputs["W_agg1"].astype(np.float32),
        "b_ag1": inputs["b_agg1"].astype(np.float32).reshape(DH, 1),
        "w_ag2": inputs["W_agg2"].astype(np.float32),
        "b2_row": inputs["b_agg2"].astype(np.float32).reshape(1, NL),
    }


def kernel(**inputs):
    inputs = {k: np.asarray(v) for k, v in inputs.items()}
    run = _get_runner()
    per_core = [_prep_core_inputs(inputs, c) for c in range(NCORES)]
    outs = run(per_core)
    label_logits = outs["o_log"].reshape(B, NL)
    label_probs = outs["o_prb"].reshape(B, NL)
    h2p = outs["o_h2p"].reshape(B, H, P)
    p2h = outs["o_p2h"].reshape(B, P, H)
    return label_logits, label_probs, h2p, p2h


# revision 14
# speedup vs baseline: 1.0155x; 1.0155x over previous
"""DecompAtt (decomposable attention) Trainium2 Bass kernel.

Full-input contract: kernel(**inputs) takes the unsharded fp32 inputs
(B=32, P=H=384, D=DA=DC=DH=512, 3 labels) and returns
(label_logits, label_probs, h2p, p2h) as full-shape np arrays.

Sharding: data-parallel over batch, 4 examples per core on 8 NeuronCores.

Numerics:
  - score path (attend FF + similarity) in tf32x2 (hi/lo split, 3 fp32r
    passes -> fp32-grade scores; measured 1.5e-7 rel on HW)
  - attention application + compare FF in single-pass fp32r (tf32)
  - aggregate MLP in true fp32 (it dominates logit error otherwise)
  - masked softmax with additive -1e30 mask folded into PSUM via K=1
    matmuls; exp on ACT with per-partition bias and fused row-sum.
"""

import numpy as np

import concourse.bacc as bacc
import concourse.mybir as mybir
import concourse.tile as tile
from concourse import masks

F32 = mybir.dt.float32
F32R = mybir.dt.float32r
AF = mybir.ActivationFunctionType
AX = mybir.AxisListType
OP = mybir.AluOpType

B, P, H, D = 32, 384, 384, 512
DA, DC, DH, NL = 512, 512, 512, 3
NCORES = 8
BE = B // NCORES  # examples per core
NEG = np.float32(-1.0e30)

_CACHE = {}


def r(ap):
    return ap.bitcast(F32R)


def _build(reps=1):
    nc = bacc.Bacc()

    # ---- per-core DRAM parameters
    emb_p = nc.declare_dram_parameter("emb_p", [BE, P, D], F32, isOutput=False)
    emb_h = nc.declare_dram_parameter("emb_h", [BE, H, D], F32, isOutput=False)
    pmadd = nc.declare_dram_parameter("pmadd", [BE, P], F32, isOutput=False)
    hmadd = nc.declare_dram_parameter("hmadd", [BE, H], F32, isOutput=False)
    pmf = nc.declare_dram_parameter("pmf", [BE, P], F32, isOutput=False)
    hmf = nc.declare_dram_parameter("hmf", [BE, H], F32, isOutput=False)
    w_att = nc.declare_dram_parameter("w_att", [D, DA], F32, isOutput=False)
    b_att = nc.declare_dram_parameter("b_att", [DA, 1], F32, isOutput=False)
    w_cmp = nc.declare_dram_parameter("w_cmp", [2 * D, DC], F32, isOutput=False)
    bc_col = nc.declare_dram_parameter("bc_col", [DC, 1], F32, isOutput=False)
    w_ag1 = nc.declare_dram_parameter("w_ag1", [2 * DC, DH], F32, isOutput=False)
    b_ag1 = nc.declare_dram_parameter("b_ag1", [DH, 1], F32, isOutput=False)
    w_ag2 = nc.declare_dram_parameter("w_ag2", [DH, NL], F32, isOutput=False)
    b2_row = nc.declare_dram_parameter("b2_row", [1, NL], F32, isOutput=False)

    o_log = nc.declare_dram_parameter("o_log", [BE, NL], F32, isOutput=True)
    o_prb = nc.declare_dram_parameter("o_prb", [BE, NL], F32, isOutput=True)
    o_h2p = nc.declare_dram_parameter("o_h2p", [BE, H, P], F32, isOutput=True)
    o_p2h = nc.declare_dram_parameter("o_p2h", [BE, P, H], F32, isOutput=True)

    TOK = P  # == H == 384; 3 token tiles of 128
    NT = TOK // 128  # 3
    ND = D // 128    # 4

    with tile.TileContext(nc) as tc:
        with (
            tc.tile_pool(name="wp", bufs=1) as wp,
            tc.tile_pool(name="xp", bufs=1) as xp,
            tc.tile_pool(name="ps_mm", bufs=3, space="PSUM") as ps_mm,
            tc.tile_pool(name="ps_tr", bufs=3, space="PSUM") as ps_tr,
            tc.tile_pool(name="ps_sm", bufs=2, space="PSUM") as ps_sm,
        ):
            _rep = tc.For_i(0, reps, 1) if reps > 1 else None
            if _rep is not None:
                _rep.__enter__()
            # ================= constants & weights =================
            ident = wp.tile([128, 128], F32, name="ident")
            masks.make_identity(nc, ident[:])

            ones_raw = wp.tile([1, 128], F32, name="ones_raw")
            nc.vector.memset(ones_raw[:], 1.0)
            ones_r = wp.tile([1, 128], F32, name="ones_r")
            nc.vector.tensor_copy(r(ones_r[:]), ones_raw[:])
            ones4 = wp.tile([1, BE], F32, name="ones4")
            nc.vector.memset(ones4[:], 1.0)

            wa_hi, wa_lo, wcr, w1t, w2t = [], [], [], [], []
            for k in range(ND):
                t = xp.tile([128, DA], F32, name=f"wa_raw{k}", tag="wa_raw", bufs=2)
                nc.sync.dma_start(out=t[:], in_=w_att[128 * k:128 * (k + 1), :])
                hi = wp.tile([128, DA], F32, name=f"wa_hi{k}")
                lo = wp.tile([128, DA], F32, name=f"wa_lo{k}")
                nc.vector.tensor_copy(r(hi[:]), t[:])
                nc.vector.tensor_sub(r(lo[:]), t[:], hi[:])
                wa_hi.append(hi)
                wa_lo.append(lo)
            for k in range(2 * D // 128):
                t = xp.tile([128, DC], F32, name=f"wc_raw{k}", tag="wa_raw", bufs=2)
                nc.sync.dma_start(out=t[:], in_=w_cmp[128 * k:128 * (k + 1), :])
                t2 = wp.tile([128, DC], F32, name=f"wc{k}")
                nc.vector.tensor_copy(r(t2[:]), t[:])
                wcr.append(t2)
            def load_w1(k):
                t = xp.tile([128, DH], F32, name=f"w1_{k}", tag="wa_raw", bufs=2)
                nc.sync.dma_start(out=t[:], in_=w_ag1[128 * k:128 * (k + 1), :])
                return t
            for k in range(DH // 128):
                t = wp.tile([128, NL], F32, name=f"w2_{k}")
                nc.sync.dma_start(out=t[:], in_=w_ag2[128 * k:128 * (k + 1), :])
                w2t.append(t)

            batt = wp.tile([128, ND], F32, name="batt")
            nc.sync.dma_start(out=batt[:], in_=b_att[:].rearrange("(j p) o -> p (j o)", p=128))
            b1c = wp.tile([128, DH // 128], F32, name="b1c")
            nc.sync.dma_start(out=b1c[:], in_=b_ag1[:].rearrange("(j p) o -> p (j o)", p=128))
            bcc = wp.tile([128, DC // 128], F32, name="bcc")
            nc.sync.dma_start(out=bcc[:], in_=bc_col[:].rearrange("(j p) o -> p (j o)", p=128))
            b2r = wp.tile([1, NL], F32, name="b2r")
            nc.sync.dma_start(out=b2r[:], in_=b2_row[:])

            # per-example additive mask rows, each at partition 0
            madd_p_e, madd_h_e = [], []
            for e in range(BE):
                tp0 = wp.tile([1, TOK], F32, name=f"maddp_raw{e}")
                nc.sync.dma_start(out=tp0[:], in_=pmadd[e:e + 1, :])
                tp = wp.tile([1, TOK], F32, name=f"maddp{e}")
                nc.vector.tensor_copy(r(tp[:]), tp0[:])
                madd_p_e.append(tp)
                th0 = wp.tile([1, TOK], F32, name=f"maddh_raw{e}")
                nc.sync.dma_start(out=th0[:], in_=hmadd[e:e + 1, :])
                th = wp.tile([1, TOK], F32, name=f"maddh{e}")
                nc.vector.tensor_copy(r(th[:]), th0[:])
                madd_h_e.append(th)


            # aggregate rhs: 8 k-tiles [128, BE]
            aggt = [wp.tile([128, BE], F32, name=f"agg{k}") for k in range(8)]

            # ================= helpers =================
            def softmax_from_psum(zp, out_sb, tag):
                """masked softmax over free dim of z (PSUM [128, TOK]) -> out_sb fp32."""
                negc = xp.tile([128, 1], F32, name=f"negc_{tag}", tag="negc", bufs=4)
                nc.vector.tensor_reduce(out=negc[:], in_=zp[:], axis=AX.X,
                                        op=OP.max, negate=True)
                ts = xp.tile([128, 1], F32, name=f"ts_{tag}", tag="ts", bufs=4)
                nc.scalar.activation(out_sb[:], zp[:], AF.Exp,
                                     bias=negc[:], scale=1.0, accum_out=ts[:])
                it = xp.tile([128, 1], F32, name=f"it_{tag}", tag="it", bufs=4)
                nc.vector.reciprocal(it[:], ts[:])
                nc.scalar.activation(out_sb[:], out_sb[:], AF.Copy, bias=0.0, scale=it[:])

            # ================= per-example pipeline =================
            for e in range(BE):
                # ---- load emb tiles (native [tok, D])
                embs = {}
                for side, src in (("p", emb_p), ("h", emb_h)):
                    tl = []
                    for t in range(NT):
                        s = xp.tile([128, D], F32, name=f"emb{side}{t}_e{e}",
                                    tag=f"emb{side}{t}", bufs=1)
                        nc.sync.dma_start(out=s[:], in_=src[e, 128 * t:128 * (t + 1), :])
                        tl.append(s)
                    embs[side] = tl

                # ---- embT (transpose) + hi/lo split : per side 4 tiles [128(d), TOK]
                ehi, elo = {}, {}
                for side in ("p", "h"):
                    his, los = [], []
                    for dj in range(ND):
                        pt = ps_tr.tile([128, TOK], F32, name=f"ptT{side}{dj}_e{e}",
                                        tag="ps_tr_big", bufs=3)
                        for ti in range(NT):
                            nc.tensor.matmul(
                                pt[:, 128 * ti:128 * (ti + 1)],
                                embs[side][ti][:, 128 * dj:128 * (dj + 1)],
                                ident[:], is_transpose=True,
                                start=(ti == 0), stop=(ti == NT - 1))
                        hi = xp.tile([128, TOK], F32, name=f"ehi{side}{dj}_e{e}",
                                     tag=f"ehi{side}{dj}", bufs=2)
                        lo = xp.tile([128, TOK], F32, name=f"elo{side}{dj}_e{e}",
                                     tag=f"elo{side}{dj}", bufs=1)
                        nc.scalar.activation(r(hi[:]), pt[:], AF.Copy)
                        nc.vector.tensor_sub(r(lo[:]), pt[:], hi[:])
                        his.append(hi)
                        los.append(lo)
                    ehi[side], elo[side] = his, los

                # ---- rounded copies of native emb (f32r) for M3 lhsT
                embr = {}
                for side in ("p", "h"):
                    rl = []
                    for t in range(NT):
                        rt = xp.tile([128, D], F32, name=f"embr{side}{t}_e{e}",
                                     tag=f"embr{side}{t}", bufs=1)
                        nc.vector.tensor_copy(r(rt[:]), embs[side][t][:])
                        rl.append(rt)
                    embr[side] = rl

                # ---- M1: projT = relu(W_att.T @ embT + b) ; tf32x2, keep hi/lo
                phi, plo = {}, {}
                for side in ("p", "h"):
                    his, los = [], []
                    for dj in range(ND):
                        pp = ps_mm.tile([128, TOK], F32, name=f"pp{side}{dj}_e{e}",
                                        tag="ps_mm_big", bufs=3)
                        sl = slice(128 * dj, 128 * (dj + 1))
                        for k in range(ND):
                            nc.tensor.matmul(pp[:], r(wa_hi[k][:, sl]), r(ehi[side][k][:]),
                                             start=(k == 0), stop=False)
                        for k in range(ND):
                            nc.tensor.matmul(pp[:], r(wa_hi[k][:, sl]), r(elo[side][k][:]),
                                             start=False, stop=False)
                        for k in range(ND):
                            nc.tensor.matmul(pp[:], r(wa_lo[k][:, sl]), r(ehi[side][k][:]),
                                             start=False, stop=(k == ND - 1))
                        fu = xp.tile([128, TOK], F32, name=f"pf{side}{dj}_e{e}",
                                     tag="proj_full", bufs=2)
                        nc.scalar.activation(fu[:], pp[:], AF.Relu,
                                             bias=batt[:, dj:dj + 1], scale=1.0)
                        hi = xp.tile([128, TOK], F32, name=f"phi{side}{dj}_e{e}",
                                     tag=f"phi{side}{dj}", bufs=1)
                        lo = xp.tile([128, TOK], F32, name=f"plo{side}{dj}_e{e}",
                                     tag=f"plo{side}{dj}", bufs=1)
                        nc.vector.tensor_copy(r(hi[:]), fu[:])
                        nc.vector.tensor_sub(r(lo[:]), fu[:], hi[:])
                        his.append(hi)
                        los.append(lo)
                    phi[side], plo[side] = his, los

                # ---- M2: sim tiles [128(p), H] tf32x2 ; raw copy; +hm mask; softmax
                sim_sb, p2h_sb = [], []
                for pi in range(NT):
                    sp = ps_mm.tile([128, TOK], F32, name=f"sim{pi}_e{e}",
                                    tag="ps_mm_big", bufs=3)
                    sl = slice(128 * pi, 128 * (pi + 1))
                    for k in range(ND):
                        nc.tensor.matmul(sp[:], r(phi["p"][k][:, sl]), r(phi["h"][k][:]),
                                         start=(k == 0), stop=False)
                    for k in range(ND):
                        nc.tensor.matmul(sp[:], r(phi["p"][k][:, sl]), r(plo["h"][k][:]),
                                         start=False, stop=False)
                    for k in range(ND):
                        nc.tensor.matmul(sp[:], r(plo["p"][k][:, sl]), r(phi["h"][k][:]),
                                         start=False, stop=False)
                    raw = xp.tile([128, TOK], F32, name=f"simsb{pi}_e{e}",
                                  tag=f"simsb{pi}", bufs=1)
                    nc.scalar.activation(raw[:], sp[:], AF.Copy)
                    sim_sb.append(raw)
                    nc.tensor.matmul(sp[:], r(ones_r[:]), r(madd_h_e[e][:]),
                                     start=False, stop=True)
                    o = xp.tile([128, TOK], F32, name=f"p2h{pi}_e{e}",
                                tag=f"p2h{pi}", bufs=1)
                    softmax_from_psum(sp, o, f"p2h{pi}e{e}")
                    nc.sync.dma_start(out=o_p2h[e, 128 * pi:128 * (pi + 1), :], in_=o[:])
                    p2h_sb.append(o)

                # ---- simT via PE transpose + pm mask; softmax -> h2p
                h2p_sb = []
                for hi_ in range(NT):
                    st = ps_tr.tile([128, TOK], F32, name=f"simT{hi_}_e{e}",
                                    tag="ps_tr_big", bufs=3)
                    for pi in range(NT):
                        nc.tensor.matmul(
                            st[:, 128 * pi:128 * (pi + 1)],
                            sim_sb[pi][:, 128 * hi_:128 * (hi_ + 1)],
                            ident[:], is_transpose=True,
                            start=(pi == 0), stop=False)
                    nc.tensor.matmul(st[:], r(ones_r[:]), r(madd_p_e[e][:]),
                                     start=False, stop=True)
                    o = xp.tile([128, TOK], F32, name=f"h2p{hi_}_e{e}",
                                tag=f"h2p{hi_}", bufs=1)
                    softmax_from_psum(st, o, f"h2p{hi_}e{e}")
                    nc.sync.dma_start(out=o_h2p[e, 128 * hi_:128 * (hi_ + 1), :], in_=o[:])
                    h2p_sb.append(o)

                # ---- transposes of p2h/h2p (f32r) for attended matmuls
                def transpose_set(src_tiles, tag):
                    outs = []
                    for j in range(NT):
                        tp = ps_tr.tile([128, TOK], F32, name=f"{tag}T{j}_e{e}",
                                        tag="ps_tr_big", bufs=3)
                        for i in range(NT):
                            nc.tensor.matmul(
                                tp[:, 128 * i:128 * (i + 1)],
                                src_tiles[i][:, 128 * j:128 * (j + 1)],
                                ident[:], is_transpose=True,
                                start=(i == 0), stop=(i == NT - 1))
                        ot = xp.tile([128, TOK], F32, name=f"{tag}{j}_e{e}",
                                     tag=f"{tag}{j}", bufs=1)
                        nc.scalar.activation(r(ot[:]), tp[:], AF.Copy)
                        outs.append(ot)
                    return outs

                p2hT = transpose_set(p2h_sb, "p2hT")  # [128(h), P] tiles
                h2pT = transpose_set(h2p_sb, "h2pT")  # [128(p), H] tiles

                # ---- M3: attended (tf32): attT_h [128(d), P] ; attT_p [128(d), H]
                def attend(embn, probT, tag):
                    outs = []
                    for dj in range(ND):
                        ap = ps_mm.tile([128, TOK], F32, name=f"{tag}{dj}_e{e}",
                                        tag="ps_mm_big", bufs=3)
                        sl = slice(128 * dj, 128 * (dj + 1))
                        for t in range(NT):
                            nc.tensor.matmul(ap[:], r(embn[t][:, sl]), r(probT[t][:]),
                                             start=(t == 0), stop=(t == NT - 1))
                        ot = xp.tile([128, TOK], F32, name=f"{tag}sb{dj}_e{e}",
                                     tag=f"{tag}{dj}", bufs=1)
                        nc.scalar.activation(r(ot[:]), ap[:], AF.Copy)
                        outs.append(ot)
                    return outs

                att_h = attend(embr["h"], p2hT, "atth")  # attended_hypothesis^T
                att_p = attend(embr["p"], h2pT, "attp")  # attended_premise^T

                # ---- M4: compare (dc-major) with -1e30 token masking folded in
                # PSUM; relu+bias+masked-sum fused into one ACT op per tile
                # (accum_out writes the compared chunk straight into aggt).
                def compare(concat_tiles, madd_row, agg_base, tag):
                    for dj in range(4):
                        cp = ps_mm.tile([128, TOK], F32, name=f"cp{tag}{dj}_e{e}",
                                        tag="ps_mm_big", bufs=3)
                        sl = slice(128 * dj, 128 * (dj + 1))
                        for k in range(8):
                            nc.tensor.matmul(cp[:], r(wcr[k][:, sl]), r(concat_tiles[k][:]),
                                             start=(k == 0), stop=False)
                        nc.tensor.matmul(cp[:], r(ones_r[:]), r(madd_row[:]),
                                         start=False, stop=True)
                        cm = xp.tile([128, TOK], F32, name=f"cm{tag}{dj}_e{e}",
                                     tag="cmp_scratch", bufs=2)
                        nc.scalar.activation(cm[:], cp[:], AF.Relu,
                                             bias=bcc[:, dj:dj + 1], scale=1.0,
                                             accum_out=aggt[agg_base + dj][:, e:e + 1])

                compare(ehi["p"] + att_h, madd_p_e[e], 0, "p")
                compare(ehi["h"] + att_p, madd_h_e[e], 4, "h")

            # ================= aggregate MLP (fp32), all examples =================
            nh = DH // 128
            hp = ps_sm.tile([128, BE * nh], F32, name="hid_all", tag="smll", bufs=2)
            for k in range(8):
                wk = load_w1(k)
                for dj in range(nh):
                    nc.tensor.matmul(hp[:, BE * dj:BE * (dj + 1)],
                                     wk[:, 128 * dj:128 * (dj + 1)], aggt[k][:],
                                     start=(k == 0 and dj == 0),
                                     stop=(k == 7 and dj == nh - 1))
            hid = []
            for dj in range(nh):
                hs = wp.tile([128, BE], F32, name=f"hids{dj}")
                nc.scalar.activation(hs[:], hp[:, BE * dj:BE * (dj + 1)], AF.Relu,
                                     bias=b1c[:, dj:dj + 1], scale=1.0)
                hid.append(hs)

            lp = ps_sm.tile([BE, NL], F32, name="logitp", tag="smll", bufs=2)
            for k in range(DH // 128):
                nc.tensor.matmul(lp[:], hid[k][:], w2t[k][:],
                                 start=(k == 0), stop=False)
            nc.tensor.matmul(lp[:], ones4[:], b2r[:], start=False, stop=True)
            lg = wp.tile([BE, NL], F32, name="lg")
            nc.vector.tensor_copy(lg[:], lp[:])
            nc.sync.dma_start(out=o_log[:], in_=lg[:])

            ng4 = wp.tile([BE, 1], F32, name="ng4")
            nc.vector.tensor_reduce(out=ng4[:], in_=lg[:], axis=AX.X, op=OP.max,
                                    negate=True)
            pr = wp.tile([BE, NL], F32, name="pr")
            t4 = wp.tile([BE, 1], F32, name="t4")
            nc.scalar.activation(pr[:], lg[:], AF.Exp, bias=ng4[:], scale=1.0,
                                 accum_out=t4[:])
            i4 = wp.tile([BE, 1], F32, name="i4")
            nc.vector.reciprocal(i4[:], t4[:])
            nc.scalar.activation(pr[:], pr[:], AF.Copy, bias=0.0, scale=i4[:])
            nc.sync.dma_start(out=o_prb[:], in_=pr[:])
            if _rep is not None:
                _rep.__exit__(None, None, None)

    nc.finalize()
    return nc


def _get_runner():
    """Build nc once, jit the sharded executor once; reuse across calls."""
    if "runner" in _CACHE:
        return _CACHE["runner"]

    import jax
    from jax.sharding import Mesh, PartitionSpec
    from jax.experimental.shard_map import shard_map
    from concourse import bass2jax

    nc = _build()
    bass2jax.install_neuronx_cc_hook()

    in_names = []
    out_names = []
    out_avals = []
    zero_outs = []
    partition_name = nc.partition_id_tensor.name if nc.partition_id_tensor else None
    for alloc in nc.m.functions[0].allocations:
        if not isinstance(alloc, mybir.MemoryLocationSet):
            continue
        name = alloc.memorylocations[0].name
        if alloc.kind == "ExternalInput":
            if name != partition_name:
                in_names.append(name)
        elif alloc.kind == "ExternalOutput":
            shape = tuple(alloc.tensor_shape)
            dtype = mybir.dt.np(alloc.dtype)
            out_names.append(name)
            out_avals.append(jax.core.ShapedArray(shape, dtype))
            zero_outs.append(np.zeros(shape, dtype))
    n_params = len(in_names)
    n_outs = len(out_names)
    all_in = in_names + out_names

    if partition_name is not None:
        all_in = all_in + [partition_name]

    def _body(*args):
        operands = list(args)
        if partition_name is not None:
            operands.append(bass2jax.partition_id_tensor())
        outs = bass2jax._bass_exec_p.bind(
            *operands,
            out_avals=tuple(out_avals),
            in_names=tuple(all_in),
            out_names=tuple(out_names),
            lowering_input_output_aliases=(),
            sim_require_finite=True,
            sim_require_nnan=True,
            nc=nc,
        )
        return tuple(outs)

    devices = jax.devices()[:NCORES]
    mesh = Mesh(np.asarray(devices), ("core",))
    in_specs = (PartitionSpec("core"),) * (n_params + n_outs)
    out_specs = (PartitionSpec("core"),) * n_outs
    donate = tuple(range(n_params, n_params + n_outs))
    sharded = jax.jit(
        shard_map(_body, mesh=mesh, in_specs=in_specs, out_specs=out_specs,
                  check_rep=False),
        donate_argnums=donate, keep_unused=True)

    def run(per_core_maps):
        concat_in = [
            np.concatenate([per_core_maps[c][n] for c in range(NCORES)], axis=0)
            for n in in_names
        ]
        concat_zero = [
            np.zeros((NCORES * z.shape[0], *z.shape[1:]), z.dtype) for z in zero_outs
        ]
        outs = sharded(*concat_in, *concat_zero)
        return {
            n: np.asarray(outs[i]).reshape(NCORES, *out_avals[i].shape)
            for i, n in enumerate(out_names)
        }

    _CACHE["nc"] = nc
    _CACHE["runner"] = run
    _CACHE["exec_parts"] = {
        "nc": nc, "in_names": in_names, "out_names": out_names,
        "out_avals": out_avals, "zero_outs": zero_outs,
        "partition_name": partition_name, "body": _body,
    }
    return run


def _prep_core_inputs(inputs, c):
    sl = slice(c * BE, (c + 1) * BE)
    ep = np.ascontiguousarray(inputs["embedded_premise"][sl]).astype(np.float32)
    eh = np.ascontiguousarray(inputs["embedded_hypothesis"][sl]).astype(np.float32)
    pm = inputs["premise_mask"][sl].astype(np.float32)
    hm = inputs["hypothesis_mask"][sl].astype(np.float32)
    return {
        "emb_p": ep,
        "emb_h": eh,
        "pmadd": np.where(pm > 0, np.float32(0), NEG).astype(np.float32),
        "hmadd": np.where(hm > 0, np.float32(0), NEG).astype(np.float32),
        "pmf": pm,
        "hmf": hm,
        "w_att": inputs["W_attend"].astype(np.float32),
        "b_att": inputs["b_attend"].astype(np.float32).reshape(DA, 1),
        "w_cmp": inputs["W_compare"].astype(np.float32),
        "bc_col": inputs["b_compare"].astype(np.float32).reshape(DC, 1),
        "w_ag1": inputs["W_agg1"].astype(np.float32),
        "b_ag1": inputs["b_agg1"].astype(np.float32).reshape(DH, 1),
        "w_ag2": inputs["W_agg2"].astype(np.float32),
        "b2_row": inputs["b_agg2"].astype(np.float32).reshape(1, NL),
    }


def kernel(**inputs):
    inputs = {k: np.asarray(v) for k, v in inputs.items()}
    run = _get_runner()
    per_core = [_prep_core_inputs(inputs, c) for c in range(NCORES)]
    outs = run(per_core)
    label_logits = outs["o_log"].reshape(B, NL)
    label_probs = outs["o_prb"].reshape(B, NL)
    h2p = outs["o_h2p"].reshape(B, H, P)
    p2h = outs["o_p2h"].reshape(B, P, H)
    return label_logits, label_probs, h2p, p2h
